# revision 1
# baseline (speedup 1.0000x reference)
# Trainium2 Bass kernel for nn_MipMap — v2: f-sharded grid build.
#
# Each core builds G[:, :, 16-feature slice] for the full 256x256 grid
# (all 6 mip levels pre-combined, conv matrices baked at trace time),
# with a PE-transpose between the axis-0 and axis-1 conv passes (all in
# SBUF, no DRAM staging).  Cores then exchange shards: AllToAll (33-row
# halo chunks) -> local DVE f-interleave into a row-pair-duplicated
# layout dupJ[j,i] = [G[j,i] | G[j+1,i]] -> AllGather the 4MB dup
# stripe -> every core holds the 32MB dup grid.  Point phase: points
# host-sorted by cell, ONE 1KB indirect gather per point (4 corners),
# bilinear combine + 2-layer MLP as before.  Host un-permutes outputs.

import numpy as np

R = 256
F = 128
FS = 16                              # features per core
FH = 8                               # features per build half
N_PTS = 262144
N_CORES = 8
PTS_CORE = N_PTS // N_CORES          # 32768
PT_COLS = PTS_CORE // 128            # 256
LEVELS = [1, 2, 3, 4, 5]
GG = 8                               # point-tiles per gather group

# banded M-tiles: (out_start, out_len); segs: (w, sbuf_tile_idx)
P_TILES = [(0, 96), (96, 96), (192, 64)]
SEGS = [(0, 0), (1, 0), (1, 1), (2, 1)]
# psum->sbuf copy splits per M-tile: (dst_tile, dst_lo, dst_hi, src_lo, src_hi)
MSPLIT = {0: [(0, 0, 96, 0, 96)],
          1: [(0, 96, 128, 0, 32), (1, 0, 32, 32, 64), (1, 32, 64, 64, 96)],
          2: [(1, 64, 128, 0, 64)]}


def _gaussian_kernel(M, std):
    n = np.arange(M, dtype=np.float64) - (M - 1) / 2.0
    w = np.exp(-0.5 * (n / std) ** 2)
    return (w / w.sum()).astype(np.float32)


def _conv_matrix(s):
    kern = _gaussian_kernel(s, s / 2.0).astype(np.float64)
    A = np.zeros((R, R), np.float64)
    for z in range(R):
        for k in range(s):
            m = z - s // 2 + k
            if m < 0:
                m = -m
            elif m > R - 1:
                m = 2 * (R - 1) - m
            A[z, m] += kern[k]
    return A


def _seg_lhsT(A):
    """[4, 128, 96]: seg lhsT[r, m] = A[o0+m, t*128+r]."""
    out = np.zeros((4, 128, 96), np.float64)
    for si, (w, t) in enumerate(SEGS):
        o0, olen = P_TILES[w]
        blk = A[o0:o0 + olen, t * 128:(t + 1) * 128]     # [olen, 128]
        out[si, :, :olen] = blk.T
    return out


def _build_consts2(b_levels):
    mats = {l: _conv_matrix(2 ** l) for l in LEVELS}
    c1 = np.stack([_seg_lhsT(mats[l]) for l in LEVELS])          # [5,4,128,96]
    d2 = np.stack([_seg_lhsT(mats[l] * float(b_levels[li + 1]))
                   for li, l in enumerate(LEVELS)])
    i2 = _seg_lhsT(np.eye(R) * float(b_levels[0]))               # [4,128,96]
    return c1.astype(np.float16), d2.astype(np.float16), i2.astype(np.float16)


def _numpy_model2(pt, X, b_levels, W1, b1, W2, b2):
    """Mirror of the device algorithm using the baked seg matrices."""
    c1, d2, i2 = _build_consts2(np.asarray(b_levels, np.float32))
    X16 = np.asarray(X, np.float16).astype(np.float32)           # [i,j,f]
    G = np.zeros((R, R, F), np.float32)                          # G^T [j,i,f]
    for li in range(5):
        # pass1: Y[i_out, j', f] via segs
        Y = np.zeros((R, R, F), np.float32)
        for si, (w, t) in enumerate(SEGS):
            o0, olen = P_TILES[w]
            lhsT = c1[li, si].astype(np.float32)                 # [128, 96]
            xin = X16[t * 128:(t + 1) * 128]                     # [128, j', f]
            Y[o0:o0 + olen] += np.einsum("km,kjf->mjf", lhsT[:, :olen], xin)
        Y = Y.astype(np.float16).astype(np.float32)
        # pass2: G[j,i,f] += sum_j' d2[j,j'] Y^T[j', i, f]
        YT = Y.transpose(1, 0, 2)                                # [j', i, f]
        for si, (w, t) in enumerate(SEGS):
            o0, olen = P_TILES[w]
            lhsT = d2[li, si].astype(np.float32)
            G[o0:o0 + olen] += np.einsum(
                "km,kif->mif", lhsT[:, :olen], YT[t * 128:(t + 1) * 128])
    XT = X16.transpose(1, 0, 2)
    for si, (w, t) in enumerate(SEGS):
        o0, olen = P_TILES[w]
        lhsT = i2[si].astype(np.float32)
        G[o0:o0 + olen] += np.einsum(
            "km,kif->mif", lhsT[:, :olen], XT[t * 128:(t + 1) * 128])
    G = G.astype(np.float16)
    # dup grid: dupJ[j, i] = [G[j,i] | G[min(j+1,255), i]]
    dup = np.zeros((R, R, 2, F), np.float16)
    dup[:, :, 0] = G
    dup[:255, :, 1] = G[1:]
    dup[255, :, 1] = G[255]
    dupf = dup.reshape(R * R, 2 * F)                             # rows 512B
    af = (np.asarray(pt, np.float32) + 1.0) * 127.5
    fr = af - np.floor(af)
    fl = np.floor(af).astype(np.int32)
    c0, c1i = fl[:, 0], fl[:, 1]
    f0, f1 = fr[:, 0], fr[:, 1]
    cell = c1i * 256 + c0
    strip = np.concatenate([dupf[cell], dupf[cell + 1]], axis=1)  # [N, 4F]
    A0, B0 = strip[:, 0:F], strip[:, F:2 * F]
    A1, B1 = strip[:, 2 * F:3 * F], strip[:, 3 * F:4 * F]
    w00 = ((1 - f1) * (1 - f0))[:, None]
    w10 = (f1 * (1 - f0))[:, None]
    w01 = ((1 - f1) * f0)[:, None]
    w11 = (f1 * f0)[:, None]
    feat = (A0 * w00 + B0 * w10 + A1 * w01 + B1 * w11).astype(np.float16)
    z = feat.astype(np.float32) @ np.asarray(W1, np.float32) + b1
    return np.maximum(z, 0.0) @ np.asarray(W2, np.float32) + b2


def _build_bass2():
    import concourse.bass as bass
    import concourse.mybir as mybir
    import concourse.tile as tile
    from concourse import bacc
    from concourse.masks import make_identity
    dep = tile.add_dep_helper

    f32 = mybir.dt.float32
    f16 = mybir.dt.float16
    i32 = mybir.dt.int32
    Alu = mybir.AluOpType
    Act = mybir.ActivationFunctionType

    nc = bacc.Bacc(num_devices=N_CORES)
    xf = nc.dram_tensor("xf", [R, R, FS], f16, kind="ExternalInput")
    xtf = nc.dram_tensor("xtf", [R, R, FS], f16, kind="ExternalInput")
    c1d = nc.dram_tensor("c1d", [5, 4, 128, 96], f16, kind="ExternalInput")
    d2d = nc.dram_tensor("d2d", [5, 4, 128, 96], f16, kind="ExternalInput")
    i2d = nc.dram_tensor("i2d", [4, 128, 96], f16, kind="ExternalInput")
    ptx = nc.dram_tensor("ptx", [PTS_CORE, 2], f32, kind="ExternalInput")
    w1d = nc.dram_tensor("w1d", [F, 128], f32, kind="ExternalInput")
    b1d = nc.dram_tensor("b1d", [128], f32, kind="ExternalInput")
    w2d = nc.dram_tensor("w2d", [128, 4], f32, kind="ExternalInput")
    b2d = nc.dram_tensor("b2d", [4], f32, kind="ExternalInput")
    outd = nc.dram_tensor("out", [PTS_CORE, 4], f32, kind="ExternalOutput")

    groups = [list(range(N_CORES))]

    with tile.TileContext(nc) as tc:
        with (
            tc.tile_pool(name="dram", bufs=1, space="DRAM") as dpool,
            tc.tile_pool(name="consts", bufs=1) as cpool,
            tc.tile_pool(name="gstage", bufs=1) as gpool,
        ):
            # ---- exchange DRAM buffers ----
            agin = dpool.tile([32, R * 2 * F], f16, name="agin", tag="agin")
            dupg = nc.dram_tensor("dupg", [8, 32, R * 2 * F], f16,
                                  addr_space="Shared")

            # ---- constants ----
            c1_sb = [[cpool.tile([128, 96], f16, name=f"c1_{li}_{s}", tag=f"c1_{li}_{s}")
                      for s in range(4)] for li in range(5)]
            d2_sb = [[cpool.tile([128, 96], f16, name=f"d2_{li}_{s}", tag=f"d2_{li}_{s}")
                      for s in range(4)] for li in range(5)]
            i2_sb = [cpool.tile([128, 96], f16, name=f"i2_{s}", tag=f"i2_{s}")
                     for s in range(4)]
            for li in range(5):
                for s in range(4):
                    nc.gpsimd.dma_start(c1_sb[li][s], c1d[li, s])
                    nc.gpsimd.dma_start(d2_sb[li][s], d2d[li, s])
            for s in range(4):
                nc.gpsimd.dma_start(i2_sb[s], i2d[s])
            w1_sb = cpool.tile([128, 128], f16, tag="w1_sb")
            nc.gpsimd.dma_start(w1_sb, w1d[:, :])
            w2_sb = cpool.tile([128, 4], f16, tag="w2_sb")
            nc.gpsimd.dma_start(w2_sb, w2d[:, :])
            b1_sb = cpool.tile([128, 1], f32, tag="b1_sb")
            nc.gpsimd.dma_start(b1_sb, b1d.ap().rearrange("(h o) -> h o", o=1))
            b2_sb = cpool.tile([128, 16], f32, tag="b2_sb")
            b2_bcast = bass.AP(tensor=b2d.ap().tensor, offset=0,
                               ap=[[0, 128], [0, 4], [1, 4]])
            nc.gpsimd.dma_start(b2_sb.rearrange("p (u c) -> p u c", c=4),
                                b2_bcast)
            ident = cpool.tile([128, 128], f16, tag="ident")
            make_identity(nc, ident)

            xf_sb = [cpool.tile([128, R * FS], f16, name=f"xf{t}", tag=f"xf{t}")
                     for t in range(2)]
            xtf_sb = [cpool.tile([128, R * FS], f16, name=f"xtf{t}", tag=f"xtf{t}")
                      for t in range(2)]
            for t in range(2):
                nc.gpsimd.dma_start(
                    xf_sb[t],
                    xf[t * 128:(t + 1) * 128].rearrange("i j f -> i (j f)"))
                nc.gpsimd.dma_start(
                    xtf_sb[t],
                    xtf[t * 128:(t + 1) * 128].rearrange("j i f -> j (i f)"))

            # ---- point-index/weight prep: overlaps the grid build ----
            wpool = tc.alloc_tile_pool(name="ptw", bufs=1)
            cellA = wpool.tile([128, PT_COLS], i32, tag="cellA")
            w00 = wpool.tile([128, PT_COLS], f32, tag="w00")
            w01 = wpool.tile([128, PT_COLS], f32, tag="w01")
            w10 = wpool.tile([128, PT_COLS], f32, tag="w10")
            w11 = wpool.tile([128, PT_COLS], f32, tag="w11")
            osb = wpool.tile([128, PT_COLS * 4], f32, tag="osb")
            with tc.tile_pool(name="prep", bufs=1) as prpool:
                pt_sb = prpool.tile([128, PT_COLS * 2], f32, tag="pt_sb")
                nc.sync.dma_start(
                    pt_sb, ptx.ap().rearrange("(p t) c -> p (t c)", p=128))
                af = prpool.tile([128, PT_COLS * 2], f32, tag="af")
                nc.vector.tensor_scalar(af, pt_sb, 1.0, 127.5,
                                        Alu.add, Alu.mult)
                il0 = prpool.tile([128, PT_COLS * 2], i32, tag="il0")
                nc.vector.tensor_copy(il0, af)
                ilf = prpool.tile([128, PT_COLS * 2], f32, tag="ilf")
                nc.vector.tensor_copy(ilf, il0)
                dd = prpool.tile([128, PT_COLS * 2], f32, tag="dd")
                nc.vector.tensor_tensor(dd, af, ilf, Alu.subtract)
                neg = prpool.tile([128, PT_COLS * 2], f32, tag="neg")
                nc.vector.tensor_scalar(neg, dd, 0.0, None, Alu.is_lt)
                fr = prpool.tile([128, PT_COLS * 2], f32, tag="fr")
                nc.vector.tensor_tensor(fr, dd, neg, Alu.add)
                flf = prpool.tile([128, PT_COLS * 2], f32, tag="flf")
                nc.vector.tensor_tensor(flf, ilf, neg, Alu.subtract)
                il = prpool.tile([128, PT_COLS * 2], i32, tag="il")
                nc.vector.tensor_copy(il, flf)
                il3 = il.rearrange("p (t c) -> p t c", c=2)
                fr3 = fr.rearrange("p (t c) -> p t c", c=2)
                nc.vector.tensor_scalar(cellA, il3[:, :, 1], 256, None,
                                        Alu.mult)
                nc.vector.tensor_tensor(cellA, cellA, il3[:, :, 0], Alu.add)
                g0 = prpool.tile([128, PT_COLS], f32, tag="g0")
                nc.vector.tensor_scalar(g0, fr3[:, :, 0], -1.0, 1.0,
                                        Alu.mult, Alu.add)
                g1 = prpool.tile([128, PT_COLS], f32, tag="g1")
                nc.vector.tensor_scalar(g1, fr3[:, :, 1], -1.0, 1.0,
                                        Alu.mult, Alu.add)
                nc.vector.tensor_tensor(w00, g1, g0, Alu.mult)
                nc.vector.tensor_tensor(w01, g1, fr3[:, :, 0], Alu.mult)
                nc.vector.tensor_tensor(w10, fr3[:, :, 1], g0, Alu.mult)
                nc.vector.tensor_tensor(w11, fr3[:, :, 1], fr3[:, :, 0],
                                        Alu.mult)

            # G stage per half [h][jt]: [128, (f8, i)]
            gsbh = [[gpool.tile([128, R * FH], f16, name=f"gsb{h}_{jt}",
                                tag=f"gsb{h}_{jt}") for jt in range(2)]
                    for h in range(2)]
            # per-half AllToAll buffers
            a2ain_h = [dpool.tile([8, 33, R * FH], f16, name=f"a2i{h}",
                                  tag=f"a2i{h}") for h in range(2)]
            a2aout_h = [dpool.tile([8, 33, R * FH], f16, name=f"a2o{h}",
                                   tag=f"a2o{h}") for h in range(2)]

            # ================= grid build: two f-halves =================
            for h in range(2):
                with (
                    tc.tile_pool(name=f"y{h}", bufs=1) as ypool,
                    tc.tile_pool(name=f"p1p{h}", bufs=4, space="PSUM") as p1p,
                ):
                    # Y[li][t]: [128, (j',f8)]; YT[li][t]: [128, (f8,i)]
                    Y = [[ypool.tile([128, R * FH], f16, name=f"y{li}_{t}", tag=f"y{li}_{t}")
                          for t in range(2)] for li in range(5)]
                    YT = [[ypool.tile([128, R * FH], f16, name=f"yt{li}_{t}", tag=f"yt{li}_{t}")
                           for t in range(2)] for li in range(5)]
                    # ---- pass1 ----
                    for li in range(5):
                        for w in range(3):
                            o0, olen = P_TILES[w]
                            segs = [s for s, (sw, _) in enumerate(SEGS)
                                    if sw == w]
                            for c4 in range(4):
                                ps = p1p.tile([128, 512], f32, tag="ps")
                                for k, s in enumerate(segs):
                                    t = SEGS[s][1]
                                    rhs = xf_sb[t].rearrange(
                                        "p (j f) -> p j f", f=FS)[
                                        :, c4 * 64:(c4 + 1) * 64,
                                        h * FH:(h + 1) * FH]
                                    nc.tensor.matmul(
                                        ps[:olen], lhsT=c1_sb[li][s][:, :olen],
                                        rhs=rhs, start=(k == 0),
                                        stop=(k == len(segs) - 1))
                                for (dt, d0, d1, s0, s1) in MSPLIT[w]:
                                    dst = Y[li][dt].rearrange(
                                        "p (j f) -> p j f", f=FH)[
                                        d0:d1, c4 * 64:(c4 + 1) * 64, :]
                                    if (li + w + c4) % 2 == 0:
                                        nc.vector.tensor_copy(dst,
                                                              ps[s0:s1])
                                    else:
                                        nc.scalar.activation(dst, ps[s0:s1],
                                                             Act.Copy)
                    # ---- transposes: Y[i,(j',f)] -> YT[j',(f,i)] ----
                    with tc.tile_pool(name=f"trp{h}", bufs=4,
                                      space="PSUM") as trp:
                        for li in range(5):
                            for fs in range(FH):
                                for it in range(2):
                                    for jt in range(2):
                                        src = Y[li][it].rearrange(
                                            "p (j f) -> p j f", f=FH)[
                                            :, jt * 128:(jt + 1) * 128, fs]
                                        tp = trp.tile([128, 128], f16,
                                                      tag="tp")
                                        nc.tensor.transpose(tp, src, ident)
                                        dst = YT[li][jt][
                                            :, fs * R + it * 128:
                                            fs * R + (it + 1) * 128]
                                        if (li + fs) % 2 == 0:
                                            nc.vector.tensor_copy(dst, tp)
                                        else:
                                            nc.scalar.activation(dst, tp,
                                                                 Act.Copy)
                    # ---- pass2 ----
                    with tc.tile_pool(name=f"p2p{h}", bufs=4,
                                      space="PSUM") as p2p:
                        for w in range(3):
                            o0, olen = P_TILES[w]
                            segs = [s for s, (sw, _) in enumerate(SEGS)
                                    if sw == w]
                            for c4 in range(4):
                                ps = p2p.tile([128, 512], f32, tag="ps2")
                                n_mm = len(segs) * 6
                                k = 0
                                for li in range(5):
                                    for s in segs:
                                        t = SEGS[s][1]
                                        rhs = YT[li][t].rearrange(
                                            "p (f i) -> p f i", i=R)[
                                            :, :, c4 * 64:(c4 + 1) * 64]
                                        nc.tensor.matmul(
                                            ps[:olen],
                                            lhsT=d2_sb[li][s][:, :olen],
                                            rhs=rhs, start=(k == 0),
                                            stop=False)
                                        k += 1
                                for s in segs:
                                    t = SEGS[s][1]
                                    rhs = xtf_sb[t].rearrange(
                                        "p (i f) -> p f i", f=FS)[
                                        :, h * FH:(h + 1) * FH,
                                        c4 * 64:(c4 + 1) * 64]
                                    k += 1
                                    nc.tensor.matmul(
                                        ps[:olen], lhsT=i2_sb[s][:, :olen],
                                        rhs=rhs, start=False,
                                        stop=(k == n_mm))
                                for (dt, d0, d1, s0, s1) in MSPLIT[w]:
                                    dst = gsbh[h][dt].rearrange(
                                        "p (f i) -> p f i", i=R)[
                                        d0:d1, :, c4 * 64:(c4 + 1) * 64]
                                    if (w + c4) % 2 == 0:
                                        nc.vector.tensor_copy(dst, ps[s0:s1])
                                    else:
                                        nc.scalar.activation(dst, ps[s0:s1],
                                                             Act.Copy)
                # chunk d = G^T rows [32d, 32d+33) of this half's f-slice
                for d in range(8):
                    lo = 32 * d
                    rows = [(lo, min(lo + 33, 256))]
                    if d == 7:
                        rows = [(224, 256), (255, 256)]   # repeat last row
                    off = 0
                    for (r0, r1) in rows:
                        ta = r0 // 128
                        tb = (r1 - 1) // 128
                        if ta == tb:
                            spans = [(ta, r0 - ta * 128, r1 - ta * 128)]
                        else:
                            spans = [(ta, r0 - ta * 128, 128),
                                     (tb, 0, r1 - 128)]
                        for (t, aa, bb) in spans:
                            nc.sync.dma_start(
                                a2ain_h[h][d, off:off + (bb - aa), :],
                                gsbh[h][t][aa:bb, :])
                            off += bb - aa
                nc.gpsimd.collective_compute(
                    "AllToAll", mybir.AluOpType.bypass,
                    replica_groups=groups,
                    ins=[a2ain_h[h].opt()], outs=[a2aout_h[h].opt()])

            # ================= exchange =================
            with tc.tile_pool(name="ex", bufs=1) as expool:
                for ihalf in range(2):
                    # a2aout_h[h][k] rows [e, e+32), i-half slice per f8
                    a2h = [[[expool.tile([32, 128 * FH], f16,
                                         name=f"a2h{k}_{e}_{hh}",
                                         tag=f"a2h{k}_{e}_{hh}")
                             for hh in range(2)]
                            for e in range(2)] for k in range(8)]
                    for k in range(8):
                        for e in range(2):
                            for hh in range(2):
                                src = a2aout_h[hh][k].rearrange(
                                    "p (f i) -> p f i", i=R)[
                                    e:e + 32, :,
                                    ihalf * 128:(ihalf + 1) * 128]
                                nc.sync.dma_start(
                                    a2h[k][e][hh].rearrange(
                                        "p (f i) -> p f i", i=128), src)
                    dsb = expool.tile([32, 32768], f16, name="dsb",
                                      tag="dsb")
                    for k in range(8):
                        for e in range(2):
                            for hh in range(2):
                                src = a2h[k][e][hh].rearrange(
                                    "p (f i) -> p i f", i=128)
                                dst = dsb.rearrange(
                                    "p (i e f) -> p i e f", e=2, f=F)[
                                    :, :, e,
                                    k * FS + hh * FH:
                                    k * FS + (hh + 1) * FH]
                                if (k + e + hh) % 2 == 0:
                                    nc.vector.tensor_copy(dst, src)
                                else:
                                    nc.scalar.activation(dst, src, Act.Copy)
                    nc.sync.dma_start(
                        agin[:, ihalf * 32768:(ihalf + 1) * 32768], dsb)
            cc2 = nc.gpsimd.collective_compute(
                "AllGather", mybir.AluOpType.bypass, replica_groups=groups,
                ins=[agin.opt()], outs=[dupg.ap().opt()])

            # flat view of dup grid: [65536 rows, 256 f16] (512B rows)
            dupflat = dupg.ap().rearrange("c j (r e) -> (c j r) e",
                                          e=2 * F)

            # ================= point phase =================
            with (
                tc.tile_pool(name="strips", bufs=2) as stpool,
                tc.tile_pool(name="feat", bufs=3) as fpool,
                tc.tile_pool(name="ptp", bufs=2, space="PSUM") as ptp,
                tc.tile_pool(name="ptt", bufs=2, space="PSUM") as ptt,
                tc.tile_pool(name="ptp4", bufs=2, space="PSUM") as ptp4,
            ):
                n_groups = PT_COLS // GG
                ph = None
                for g in range(n_groups):
                    t0g = g * GG
                    sts = []
                    for s_i in range(GG):
                        stt_ = stpool.tile([128, 4 * F], f16, name="st",
                                           tag=f"st{s_i}")
                        gi = nc.gpsimd.indirect_dma_start(
                            out=stt_[:, :], out_offset=None,
                            in_=dupflat[:, :],
                            in_offset=bass.IndirectOffsetOnAxis(
                                ap=cellA[:, t0g + s_i:t0g + s_i + 1],
                                axis=0))
                        sts.append(stt_)
                        if g == 0 and s_i == 0:
                            dep(gi.ins, cc2.ins,
                                reason="gathers read AllGathered dup grid")
                    for s in range(GG):
                        t = t0g + s
                        st3 = sts[s].rearrange("p (o f) -> p o f", f=F)
                        fa = fpool.tile([128, 128], f16, tag="fa")
                        nc.scalar.activation(fa, st3[:, 0, :], Act.Copy,
                                             scale=w00[:, t:t + 1])
                        fb = fpool.tile([128, 128], f16, tag="fb")
                        nc.vector.scalar_tensor_tensor(
                            out=fb, in0=st3[:, 1, :],
                            scalar=w10[:, t:t + 1], in1=fa,
                            op0=Alu.mult, op1=Alu.add)
                        fc = fpool.tile([128, 128], f16, tag="fc")
                        if s % 2 == 0:
                            nc.scalar.activation(fc, st3[:, 2, :],
                                                 Act.Copy,
                                                 scale=w01[:, t:t + 1])
                        else:
                            nc.vector.tensor_scalar(fc, st3[:, 2, :],
                                                    w01[:, t:t + 1], None,
                                                    Alu.mult)
                        fd = fpool.tile([128, 128], f16, tag="fd")
                        nc.vector.scalar_tensor_tensor(
                            out=fd, in0=st3[:, 3, :],
                            scalar=w11[:, t:t + 1], in1=fc,
                            op0=Alu.mult, op1=Alu.add)
                        feat = fpool.tile([128, 128], f16, tag="feat")
                        nc.vector.tensor_tensor(feat, fb, fd, Alu.add)
                        tp = ptt.tile([128, 128], f16, tag="tp")
                        nc.tensor.transpose(tp, feat, ident)
                        ftT = fpool.tile([128, 128], f16, tag="ftT")
                        if s % 2 == 0:
                            nc.vector.tensor_copy(ftT, tp)
                        else:
                            nc.scalar.activation(ftT, tp, Act.Copy)
                        if s % 4 == 0:
                            ph = ptp.tile([128, 512], f32, tag="ph")
                        nc.tensor.matmul(
                            ph[:, (s % 4) * 128:(s % 4 + 1) * 128],
                            lhsT=w1_sb, rhs=ftT, start=True, stop=True)
                        if s % 4 == 3:
                            h1 = fpool.tile([128, 512], f16, tag="h1")
                            nc.scalar.activation(h1, ph, Act.Relu,
                                                 bias=b1_sb[:, 0:1])
                            po = ptp4.tile([128, 16], f32, tag="po")
                            for u in range(4):
                                nc.tensor.matmul(
                                    po[:, u * 4:(u + 1) * 4],
                                    lhsT=h1[:, u * 128:(u + 1) * 128],
                                    rhs=w2_sb, start=True, stop=True)
                            nc.vector.scalar_tensor_tensor(
                                out=osb[:, (t - 3) * 4:(t + 1) * 4],
                                in0=po, scalar=1.0, in1=b2_sb,
                                op0=Alu.mult, op1=Alu.add)
                nc.sync.dma_start(
                    outd.ap().rearrange("(p t) c -> p (t c)", p=128), osb)
            wpool.release()
    nc.compile()
    return nc


def kernel(pt, base_features, b_levels, W1, b1, W2, b2):
    from concourse.bass_utils import run_bass_kernel_spmd

    pt = np.ascontiguousarray(np.asarray(pt, np.float32))
    X16 = np.asarray(base_features, np.float16)
    b_levels = np.asarray(b_levels, np.float32)
    c1, d2, i2 = _build_consts2(b_levels)

    af = (pt.astype(np.float64) + 1.0) * 127.5
    fl = np.floor(af).astype(np.int64)
    key = fl[:, 1] * 256 + fl[:, 0]
    sort_idx = np.argsort(key, kind="stable")
    pts = pt[sort_idx]

    XT16 = np.ascontiguousarray(X16.transpose(1, 0, 2))
    nc = _build_bass2()
    base = {
        "c1d": c1, "d2d": d2, "i2d": i2,
        "w1d": np.ascontiguousarray(np.asarray(W1, np.float32)),
        "b1d": np.ascontiguousarray(np.asarray(b1, np.float32)),
        "w2d": np.ascontiguousarray(np.asarray(W2, np.float32)),
        "b2d": np.ascontiguousarray(np.asarray(b2, np.float32)),
    }
    in_maps = []
    for c in range(N_CORES):
        m = dict(base)
        m["xf"] = np.ascontiguousarray(X16[:, :, c * FS:(c + 1) * FS])
        m["xtf"] = np.ascontiguousarray(XT16[:, :, c * FS:(c + 1) * FS])
        m["ptx"] = np.ascontiguousarray(pts[c * PTS_CORE:(c + 1) * PTS_CORE])
        in_maps.append(m)

    res = run_bass_kernel_spmd(nc, in_maps, core_ids=list(range(N_CORES)))
    sorted_out = np.concatenate([r["out"] for r in res.results], axis=0)
    out = np.empty_like(sorted_out)
    out[sort_idx] = sorted_out
    return out

